# revision 1
# baseline (speedup 1.0000x reference)
"""BipartiteResMRConv on 8 Trainium2 NeuronCores (Bass/Tile).

Math: out = x_dst + LeakyReLU(concat([x_dst, maxes]) @ W + b), where
maxes[d] = max over edges (s,d) of (x_dst[d] - x_src[s]) = x_dst[d] - segmin[d],
segmin[d] = min over edges of x_src[s]  (empty d -> maxes = 0).

Sharding: dsts are partitioned across 8 cores (12500 each). Per core, dsts are
sorted by degree (descending) into 12544 slots; slot j lives at SBUF partition
j%128, word j//128 of a [128, 98*128] f32 accumulator (initialized to +BIG).

The gather uses the custom SWDGE dma_gather instruction (int16 indices, fast
CounterMachine descriptor generation) instead of per-round indirect DMAs.
Because idx is int16 (<=32767) and rows are 512B, x_src is split into 4 chunks
of 25000 rows; each chunk's stripe in a host-built table gets one extra +BIG
padding row. Columns of 128 gathered rows are chunk-pure: for chunk k, round r,
word w (with R[w,k] > r), partition p receives the r-th edge of slot (w,p)
whose src lies in chunk k (repeating an edge or pointing at the +BIG row as
padding). Gathered column tiles are min-folded into the accumulator with wide
DVE ops covering runs of consecutive words.

The accumulator is PE-transposed to feature-major, combined with the
host-pre-transposed x_dst, pushed through the 2-tile fp32 matmul (W resident),
LeakyReLU+bias on ACT, residual add on DVE, and written out feature-major.
The host inverse-permutes the output and patches degree-0 dsts exactly.
"""
import numpy as np
from contextlib import ExitStack

import jax
from jax.sharding import Mesh, PartitionSpec
from jax.experimental.shard_map import shard_map

from concourse import bass, bacc, tile, mybir
from concourse.bass2jax import install_neuronx_cc_hook, _bass_exec_p, partition_id_tensor
from concourse.masks import make_identity

N_SRC = 100000
N_DST = 100000
N_EDGES = 800000
D = 128
N_CORES = 8
DST_PER_CORE = N_DST // N_CORES          # 12500
SLOTS = 12544                            # ceil(12500/128)*128
WORDS = SLOTS // 128                     # 98
LEAKY = 0.01
CHUNK_W = 4                              # words per MLP chunk (512 dsts)
NCH = 4                                  # src chunks (int16 idx limit)
CH = N_SRC // NCH                        # 25000 rows per chunk
CHP = CH + 1                             # stripe rows incl. +BIG pad row
TCOLS = 8                                # gathered columns per dma_gather call
# (8*128 = 1024 idxs -> 65 descriptors per SDMA engine, under the ~128-desc
#  per-engine ring capacity; larger calls serialize descriptor generation
#  against draining at HBM round-trip latency and run ~10x slower)
BIG = np.float32(1e30)


def _schedule(Rwk):
    """Column schedule from per-(word, chunk) round counts [WORDS, NCH].

    Returns cols [(k, w, r)], calls [(k, col0, ncols)], runs_per_call
    [[(j_in_call, w0, nwords)]].
    """
    cols = []
    calls = []
    for k in range(NCH):
        c0k = len(cols)
        Rk = Rwk[:, k]
        rmax = int(Rk.max()) if len(Rk) else 0
        for r in range(rmax):
            for w in range(WORDS):
                if Rk[w] > r:
                    cols.append((k, w, r))
        nk = len(cols) - c0k
        for t in range(0, nk, TCOLS):
            calls.append((k, c0k + t, min(TCOLS, nk - t)))
    runs_per_call = []
    for (k, c0, n) in calls:
        runs = []
        j = 0
        while j < n:
            w0 = cols[c0 + j][1]
            m = 1
            while j + m < n and cols[c0 + j + m][1] == w0 + m:
                m += 1
            runs.append((j, w0, m))
            j += m
        runs_per_call.append(runs)
    return cols, calls, runs_per_call


def _build_program(Rwk):
    cols, calls, runs_per_call = _schedule(Rwk)
    ncol = len(cols)
    nc = bacc.Bacc("TRN2", target_bir_lowering=False, debug=False,
                   num_devices=N_CORES, dynamic_dma_scratch_size=32768)
    f32 = mybir.dt.float32
    xtab = nc.dram_tensor("xtab", [NCH * CHP, D], f32, kind="ExternalInput").ap()
    xdT = nc.dram_tensor("xdT", [D, SLOTS], f32, kind="ExternalInput").ap()
    idx = nc.dram_tensor("idx", [128, max(ncol * 8, 8)], mybir.dt.int16,
                         kind="ExternalInput").ap()
    w_in = nc.dram_tensor("w_in", [2 * D, D], f32, kind="ExternalInput").ap()
    b_in = nc.dram_tensor("b_in", [D, 1], f32, kind="ExternalInput").ap()
    outT = nc.dram_tensor("outT", [D, SLOTS], f32, kind="ExternalOutput").ap()

    with tile.TileContext(nc) as tc, ExitStack() as ctx:
        pool = ctx.enter_context(tc.tile_pool(name="pool", bufs=1))
        ipool = ctx.enter_context(tc.tile_pool(name="ipool", bufs=8))
        gpool = ctx.enter_context(tc.tile_pool(name="gpool", bufs=8))
        cpool = ctx.enter_context(tc.tile_pool(name="cpool", bufs=3))
        tpsum = ctx.enter_context(tc.tile_pool(name="tpsum", bufs=3, space="PSUM"))
        mpsum = ctx.enter_context(tc.tile_pool(name="mpsum", bufs=3, space="PSUM"))

        xdT_t = pool.tile([D, SLOTS], f32)
        nc.sync.dma_start(out=xdT_t[:], in_=xdT[:])
        wa = pool.tile([D, D], f32)
        nc.sync.dma_start(out=wa[:], in_=w_in[0:D, :])
        wb = pool.tile([D, D], f32)
        nc.sync.dma_start(out=wb[:], in_=w_in[D:2 * D, :])
        b_t = pool.tile([D, 1], f32)
        nc.sync.dma_start(out=b_t[:], in_=b_in[:])
        ident = pool.tile([128, 128], f32)
        make_identity(nc, ident[:])

        acc = pool.tile([128, SLOTS], f32)
        nc.vector.memset(acc[:], float(BIG))

        for (kk, c0, n), runs in zip(calls, runs_per_call):
            idx_t = ipool.tile([128, TCOLS * 8], mybir.dt.int16, tag="idx")
            nc.sync.dma_start(out=idx_t[:, :n * 8],
                              in_=idx[:, c0 * 8:(c0 + n) * 8])
            g = gpool.tile([128, TCOLS, D], f32, tag="g")
            nc.gpsimd.dma_gather(
                g[:, :n, :], xtab[kk * CHP:(kk + 1) * CHP, :],
                idx_t[:, :n * 8], n * 128, n * 128, D,
                single_packet=False)  # HW packets cap at 64 descs/engine
            for (j, w0, m) in runs:
                sl = slice(w0 * 128, (w0 + m) * 128)
                nc.vector.tensor_tensor(
                    out=acc[:, sl], in0=acc[:, sl],
                    in1=g[:, j:j + m, :].rearrange("p a b -> p (a b)"),
                    op=mybir.AluOpType.min)

        # MLP in chunks of CHUNK_W words (512 dst columns)
        for c in range(WORDS // CHUNK_W + (1 if WORDS % CHUNK_W else 0)):
            w0 = c * CHUNK_W
            nwc = min(CHUNK_W, WORDS - w0)
            ncolw = nwc * 128
            csl = slice(w0 * 128, w0 * 128 + ncolw)
            accT = tpsum.tile([128, CHUNK_W * 128], f32, space="PSUM", tag="accT")
            for i in range(nwc):
                nc.tensor.transpose(
                    out=accT[:, i * 128:(i + 1) * 128],
                    in_=acc[:, (w0 + i) * 128:(w0 + i + 1) * 128],
                    identity=ident[:])
            maxT = cpool.tile([128, CHUNK_W * 128], f32, tag="maxT")
            nc.vector.tensor_tensor(out=maxT[:, :ncolw], in0=xdT_t[:, csl],
                                    in1=accT[:, :ncolw], op=mybir.AluOpType.subtract)
            hp = mpsum.tile([128, CHUNK_W * 128], f32, space="PSUM", tag="hp")
            nc.tensor.matmul(out=hp[:, :ncolw], lhsT=wa[:], rhs=xdT_t[:, csl],
                             start=True, stop=False)
            nc.tensor.matmul(out=hp[:, :ncolw], lhsT=wb[:], rhs=maxT[:, :ncolw],
                             start=False, stop=True)
            h = cpool.tile([128, CHUNK_W * 128], f32, tag="h")
            nc.scalar.activation(out=h[:, :ncolw], in_=hp[:, :ncolw],
                                 func=mybir.ActivationFunctionType.Lrelu,
                                 bias=b_t[:], scale=1.0, alpha=LEAKY)
            res = cpool.tile([128, CHUNK_W * 128], f32, tag="res")
            nc.vector.tensor_tensor(out=res[:, :ncolw], in0=xdT_t[:, csl],
                                    in1=h[:, :ncolw], op=mybir.AluOpType.add)
            nc.sync.dma_start(out=outT[:, csl], in_=res[:, :ncolw])
    nc.compile()
    return nc


def _run_spmd(nc, in_maps):
    install_neuronx_cc_hook()
    partition_name = nc.partition_id_tensor.name if nc.partition_id_tensor else None
    in_names, out_names, out_avals, zero_outs = [], [], [], []
    for alloc in nc.m.functions[0].allocations:
        if not isinstance(alloc, mybir.MemoryLocationSet):
            continue
        name = alloc.memorylocations[0].name
        if alloc.kind == "ExternalInput":
            if name != partition_name:
                in_names.append(name)
        elif alloc.kind == "ExternalOutput":
            shape = tuple(alloc.tensor_shape)
            dtype = mybir.dt.np(alloc.dtype)
            out_names.append(name)
            out_avals.append(jax.core.ShapedArray(shape, dtype))
            zero_outs.append(np.zeros(shape, dtype))
    n_params = len(in_names)
    n_outs = len(out_avals)
    all_in = list(in_names) + list(out_names)
    if partition_name is not None:
        all_in.append(partition_name)

    def _body(*args):
        operands = list(args)
        if partition_name is not None:
            operands.append(partition_id_tensor())
        return tuple(_bass_exec_p.bind(
            *operands, out_avals=tuple(out_avals), in_names=tuple(all_in),
            out_names=tuple(out_names), lowering_input_output_aliases=(),
            sim_require_finite=True, sim_require_nnan=True, nc=nc))

    devices = jax.devices()[:N_CORES]
    mesh = Mesh(np.asarray(devices), ("core",))
    fn = jax.jit(
        shard_map(_body, mesh=mesh,
                  in_specs=(PartitionSpec("core"),) * (n_params + n_outs),
                  out_specs=(PartitionSpec("core"),) * n_outs,
                  check_rep=False),
        keep_unused=True)
    concat_in = [np.concatenate([np.asarray(m[n]) for m in in_maps], axis=0)
                 for n in in_names]
    concat_zero = [np.zeros((N_CORES * z.shape[0], *z.shape[1:]), z.dtype)
                   for z in zero_outs]
    outs = fn(*concat_in, *concat_zero)
    return [
        {n: np.asarray(outs[i]).reshape(N_CORES, *out_avals[i].shape)[c]
         for i, n in enumerate(out_names)}
        for c in range(N_CORES)
    ], fn, concat_in, concat_zero, out_names, out_avals


def _prepare(x_src, x_dst, e, W, b):
    """Host-side sharding prep. Returns per-core in_maps + assembly info."""
    src = e[0].astype(np.int64)
    dst = e[1].astype(np.int64)
    order = np.lexsort((src, dst))       # by dst, then src (chunks contiguous)
    src_s = src[order]
    dst_s = dst[order]
    ck = src_s // CH
    inch = (src_s % CH).astype(np.int16)
    cnt4 = np.bincount(dst_s * NCH + ck, minlength=N_DST * NCH) \
        .reshape(N_DST, NCH)
    start4 = np.concatenate([[0], np.cumsum(cnt4.ravel())])[:-1] \
        .reshape(N_DST, NCH)
    deg_all = cnt4.sum(axis=1)

    # padded striped src table (shared by all cores)
    xtab = np.empty((NCH * CHP, D), dtype=np.float32)
    for k in range(NCH):
        xtab[k * CHP:k * CHP + CH] = x_src[k * CH:(k + 1) * CH]
        xtab[k * CHP + CH] = BIG

    cores = []
    Rwk = np.zeros((WORDS, NCH), dtype=np.int64)
    for c in range(N_CORES):
        base = c * DST_PER_CORE
        # sort slots by max per-chunk degree: minimizes sum_k max_p deg_k
        # within each word (43% fewer gather columns than total-degree sort)
        pi = np.argsort(-cnt4[base:base + DST_PER_CORE].max(axis=1),
                        kind="stable")
        gdst = base + pi
        cnt_s = np.zeros((SLOTS, NCH), dtype=np.int64)
        cnt_s[:DST_PER_CORE] = cnt4[gdst]
        st_s = np.zeros((SLOTS, NCH), dtype=np.int64)
        st_s[:DST_PER_CORE] = start4[gdst]
        Rwk = np.maximum(Rwk, cnt_s.reshape(WORDS, 128, NCH).max(axis=1))
        cores.append(dict(base=base, pi=pi, cnt_s=cnt_s, st_s=st_s))

    cols, calls, runs = _schedule(Rwk)
    ncol = len(cols)
    p128 = np.arange(128)

    in_maps = []
    for c in range(N_CORES):
        cc = cores[c]
        cnt_s, st_s = cc["cnt_s"], cc["st_s"]
        idx_lin = np.empty(ncol * 128, dtype=np.int16)
        j = 0
        for k in range(NCH):
            Rk = Rwk[:, k]
            rmax = int(Rk.max()) if len(Rk) else 0
            for r in range(rmax):
                ws = np.where(Rk > r)[0]
                if ws.size == 0:
                    continue
                slots = (ws[:, None] * 128 + p128[None, :]).ravel()
                cnt = cnt_s[slots, k]
                st = st_s[slots, k]
                rr = np.minimum(r, np.maximum(cnt - 1, 0))
                pos = np.minimum(st + rr, len(inch) - 1)
                val = np.where(cnt > 0, inch[pos], np.int16(CH))
                idx_lin[j * 128:(j + ws.size) * 128] = val.astype(np.int16)
                j += ws.size
        assert j == ncol
        wrapped = np.tile(idx_lin.reshape(-1, 16).T, (8, 1))
        xdT = np.zeros((D, SLOTS), dtype=np.float32)
        xdT[:, :DST_PER_CORE] = x_dst[cc["base"] + cc["pi"][:DST_PER_CORE]].T
        in_maps.append({
            "xtab": xtab,
            "xdT": xdT,
            "idx": np.ascontiguousarray(wrapped),
            "w_in": np.ascontiguousarray(W),
            "b_in": np.ascontiguousarray(b.reshape(D, 1)),
        })
    return in_maps, cores, Rwk, deg_all


_CACHE = {}
_LAST = None  # (fn, concat_in, concat_zero) from the most recent call


def kernel(x_src, x_dst, e, W, b):
    x_src = np.asarray(x_src, dtype=np.float32)
    x_dst = np.asarray(x_dst, dtype=np.float32)
    e = np.asarray(e)
    W = np.asarray(W, dtype=np.float32)
    b = np.asarray(b, dtype=np.float32)

    in_maps, cores, Rwk, deg_all = _prepare(x_src, x_dst, e, W, b)

    key = Rwk.tobytes()
    if key not in _CACHE:
        _CACHE[key] = _build_program(Rwk)
    nc = _CACHE[key]

    results, fn, ci, cz, on, oa = _run_spmd(nc, in_maps)
    global _LAST
    _LAST = (fn, ci, cz)

    out = np.empty((N_DST, D), dtype=np.float32)
    for c in range(N_CORES):
        cc = cores[c]
        base, pi = cc["base"], cc["pi"]
        outT = results[c]["outT"]                     # [D, SLOTS]
        out[base + pi[:DST_PER_CORE]] = outT[:, :DST_PER_CORE].T

    # exact host patch for degree-0 dsts (empty segments -> maxes = 0)
    z = np.where(deg_all == 0)[0]
    if z.size:
        h = x_dst[z] @ W[:D] + b
        h = np.where(h > 0, h, LEAKY * h)
        out[z] = x_dst[z] + h
    return out



# revision 12
# speedup vs baseline: 264.6768x; 264.6768x over previous
"""BipartiteResMRConv on 8 Trainium2 NeuronCores (Bass/Tile).

Math: out = x_dst + LeakyReLU(concat([x_dst, maxes]) @ W + b), where
maxes[d] = max over edges (s,d) of (x_dst[d] - x_src[s]) = x_dst[d] - segmin[d],
segmin[d] = min over edges of x_src[s]  (empty d -> maxes = 0).

Sharding: dsts are partitioned across 8 cores (12500 each). Per core, dsts are
sorted by degree (descending) into 12544 slots; slot j lives at SBUF partition
j%128, word j//128 of a [128, 98*128] f32 accumulator (initialized to +BIG).

The gather uses the custom SWDGE dma_gather instruction (int16 indices, fast
CounterMachine descriptor generation) instead of per-round indirect DMAs.
Because idx is int16 (<=32767) and rows are 512B, x_src is split into 4 chunks
of 25000 rows; each chunk's stripe in a host-built table gets one extra +BIG
padding row. Columns of 128 gathered rows are chunk-pure: for chunk k, round r,
word w (with R[w,k] > r), partition p receives the r-th edge of slot (w,p)
whose src lies in chunk k (repeating an edge or pointing at the +BIG row as
padding). Gathered column tiles are min-folded into the accumulator with wide
DVE ops covering runs of consecutive words.

The accumulator is PE-transposed to feature-major, combined with the
host-pre-transposed x_dst, pushed through the 2-tile fp32 matmul (W resident),
LeakyReLU+bias on ACT, residual add on DVE, and written out feature-major.
The host inverse-permutes the output and patches degree-0 dsts exactly.
"""
import numpy as np
from contextlib import ExitStack

import jax
from jax.sharding import Mesh, PartitionSpec
from jax.experimental.shard_map import shard_map

from concourse import bass, bacc, tile, mybir
from concourse.bass2jax import install_neuronx_cc_hook, _bass_exec_p, partition_id_tensor
from concourse.masks import make_identity

N_SRC = 100000
N_DST = 100000
N_EDGES = 800000
D = 128
N_CORES = 8
DST_PER_CORE = N_DST // N_CORES          # 12500
SLOTS = 12544                            # ceil(12500/128)*128
WORDS = SLOTS // 128                     # 98
LEAKY = 0.01
CHUNK_W = 4                              # words per MLP chunk (512 dsts)
NCH = 4                                  # src chunks (int16 idx limit)
CH = N_SRC // NCH                        # 25000 rows per chunk
CHP = CH + 1                             # stripe rows incl. +BIG pad row
TCOLS = 8                                # gathered columns per dma_gather call
# (8*128 = 1024 idxs -> 65 descriptors per SDMA engine, under the ~128-desc
#  per-engine ring capacity; larger calls serialize descriptor generation
#  against draining at HBM round-trip latency and run ~10x slower)
BIG = np.float32(1e30)
NQUEUES = 1                              # SWDGE queues (1-4); gathers round-robin


def _schedule(Rwk):
    """Column schedule from per-(word, chunk) round counts [WORDS, NCH].

    Returns cols [(k, w, r)], calls [(k, col0, ncols)], runs_per_call
    [[(j_in_call, w0, nwords)]].
    """
    cols = []
    calls = []
    for k in range(NCH):
        c0k = len(cols)
        Rk = Rwk[:, k]
        rmax = int(Rk.max()) if len(Rk) else 0
        for r in range(rmax):
            for w in range(WORDS):
                if Rk[w] > r:
                    cols.append((k, w, r))
        nk = len(cols) - c0k
        for t in range(0, nk, TCOLS):
            calls.append((k, c0k + t, min(TCOLS, nk - t)))
    runs_per_call = []
    for (k, c0, n) in calls:
        runs = []
        j = 0
        while j < n:
            w0 = cols[c0 + j][1]
            m = 1
            while j + m < n and cols[c0 + j + m][1] == w0 + m:
                m += 1
            runs.append((j, w0, m))
            j += m
        runs_per_call.append(runs)
    return cols, calls, runs_per_call


def _build_program(Rwk, loops=1):
    """Build the bass program. loops>1 wraps the whole per-iteration body in
    a hardware For_i loop (each iteration is a complete kernel execution,
    separated by the loop's all-engine barrier) — used by test.py to amortize
    the dispatch round-trip when timing."""
    cols, calls, runs_per_call = _schedule(Rwk)
    ncol = len(cols)
    nc = bacc.Bacc("TRN2", target_bir_lowering=False, debug=False,
                   num_devices=N_CORES, dynamic_dma_scratch_size=32768,
                   num_swdge_queues=NQUEUES)
    f32 = mybir.dt.float32
    xtab = nc.dram_tensor("xtab", [NCH * CHP, D], f32, kind="ExternalInput").ap()
    xdT = nc.dram_tensor("xdT", [D, SLOTS], f32, kind="ExternalInput").ap()
    idx = nc.dram_tensor("idx", [128, max(ncol * 8, 8)], mybir.dt.int16,
                         kind="ExternalInput").ap()
    w_in = nc.dram_tensor("w_in", [2 * D, D], f32, kind="ExternalInput").ap()
    b_in = nc.dram_tensor("b_in", [D, 1], f32, kind="ExternalInput").ap()
    outT = nc.dram_tensor("outT", [D, SLOTS], f32, kind="ExternalOutput").ap()

    with tile.TileContext(nc) as tc, ExitStack() as ctx:
        pool = ctx.enter_context(tc.tile_pool(name="pool", bufs=1))
        ipool = ctx.enter_context(tc.tile_pool(name="ipool", bufs=8))
        gpool = ctx.enter_context(tc.tile_pool(name="gpool", bufs=8))
        cpool = ctx.enter_context(tc.tile_pool(name="cpool", bufs=3))
        tpsum = ctx.enter_context(tc.tile_pool(name="tpsum", bufs=3, space="PSUM"))
        mpsum = ctx.enter_context(tc.tile_pool(name="mpsum", bufs=3, space="PSUM"))

        xdT_t = pool.tile([D, SLOTS], f32)
        nc.sync.dma_start(out=xdT_t[:], in_=xdT[:])
        wa = pool.tile([D, D], f32)
        nc.sync.dma_start(out=wa[:], in_=w_in[0:D, :])
        wb = pool.tile([D, D], f32)
        nc.sync.dma_start(out=wb[:], in_=w_in[D:2 * D, :])
        b_t = pool.tile([D, 1], f32)
        nc.sync.dma_start(out=b_t[:], in_=b_in[:])
        ident = pool.tile([128, 128], f32)
        make_identity(nc, ident[:])
        acc = pool.tile([128, SLOTS], f32)

        def body():
            nc.vector.memset(acc[:], float(BIG))

            for ci, ((kk, c0, n), runs) in enumerate(zip(calls, runs_per_call)):
                idx_t = ipool.tile([128, TCOLS * 8], mybir.dt.int16, tag="idx")
                nc.sync.dma_start(out=idx_t[:, :n * 8],
                                  in_=idx[:, c0 * 8:(c0 + n) * 8])
                g = gpool.tile([128, TCOLS, D], f32, tag="g")
                nc.gpsimd.dma_gather(
                    g[:, :n, :], xtab[kk * CHP:(kk + 1) * CHP, :],
                    idx_t[:, :n * 8], n * 128, n * 128, D,
                    single_packet=False,  # HW packets cap at 64 descs/engine
                    queue_num=ci % NQUEUES)
                for (j, w0, m) in runs:
                    sl = slice(w0 * 128, (w0 + m) * 128)
                    nc.vector.tensor_tensor(
                        out=acc[:, sl], in0=acc[:, sl],
                        in1=g[:, j:j + m, :].rearrange("p a b -> p (a b)"),
                        op=mybir.AluOpType.min)

            # MLP in chunks of CHUNK_W words (512 dst columns)
            for c in range(WORDS // CHUNK_W + (1 if WORDS % CHUNK_W else 0)):
                w0 = c * CHUNK_W
                nwc = min(CHUNK_W, WORDS - w0)
                ncolw = nwc * 128
                csl = slice(w0 * 128, w0 * 128 + ncolw)
                accT = tpsum.tile([128, CHUNK_W * 128], f32, space="PSUM",
                                  tag="accT")
                for i in range(nwc):
                    nc.tensor.transpose(
                        out=accT[:, i * 128:(i + 1) * 128],
                        in_=acc[:, (w0 + i) * 128:(w0 + i + 1) * 128],
                        identity=ident[:])
                maxT = cpool.tile([128, CHUNK_W * 128], f32, tag="maxT")
                nc.vector.tensor_tensor(out=maxT[:, :ncolw], in0=xdT_t[:, csl],
                                        in1=accT[:, :ncolw],
                                        op=mybir.AluOpType.subtract)
                hp = mpsum.tile([128, CHUNK_W * 128], f32, space="PSUM", tag="hp")
                nc.tensor.matmul(out=hp[:, :ncolw], lhsT=wa[:], rhs=xdT_t[:, csl],
                                 start=True, stop=False)
                nc.tensor.matmul(out=hp[:, :ncolw], lhsT=wb[:],
                                 rhs=maxT[:, :ncolw], start=False, stop=True)
                h = cpool.tile([128, CHUNK_W * 128], f32, tag="h")
                nc.scalar.activation(out=h[:, :ncolw], in_=hp[:, :ncolw],
                                     func=mybir.ActivationFunctionType.Lrelu,
                                     bias=b_t[:], scale=1.0, alpha=LEAKY)
                res = cpool.tile([128, CHUNK_W * 128], f32, tag="res")
                nc.vector.tensor_tensor(out=res[:, :ncolw], in0=xdT_t[:, csl],
                                        in1=h[:, :ncolw], op=mybir.AluOpType.add)
                nc.sync.dma_start(out=outT[:, csl], in_=res[:, :ncolw])

        if loops > 1:
            with tc.For_i(0, loops, 1):
                body()
        else:
            body()
    nc.compile()
    return nc


def _run_spmd(nc, in_maps):
    install_neuronx_cc_hook()
    partition_name = nc.partition_id_tensor.name if nc.partition_id_tensor else None
    in_names, out_names, out_avals, zero_outs = [], [], [], []
    for alloc in nc.m.functions[0].allocations:
        if not isinstance(alloc, mybir.MemoryLocationSet):
            continue
        name = alloc.memorylocations[0].name
        if alloc.kind == "ExternalInput":
            if name != partition_name:
                in_names.append(name)
        elif alloc.kind == "ExternalOutput":
            shape = tuple(alloc.tensor_shape)
            dtype = mybir.dt.np(alloc.dtype)
            out_names.append(name)
            out_avals.append(jax.core.ShapedArray(shape, dtype))
            zero_outs.append(np.zeros(shape, dtype))
    n_params = len(in_names)
    n_outs = len(out_avals)
    all_in = list(in_names) + list(out_names)
    if partition_name is not None:
        all_in.append(partition_name)

    def _body(*args):
        operands = list(args)
        if partition_name is not None:
            operands.append(partition_id_tensor())
        return tuple(_bass_exec_p.bind(
            *operands, out_avals=tuple(out_avals), in_names=tuple(all_in),
            out_names=tuple(out_names), lowering_input_output_aliases=(),
            sim_require_finite=True, sim_require_nnan=True, nc=nc))

    devices = jax.devices()[:N_CORES]
    mesh = Mesh(np.asarray(devices), ("core",))
    fn = jax.jit(
        shard_map(_body, mesh=mesh,
                  in_specs=(PartitionSpec("core"),) * (n_params + n_outs),
                  out_specs=(PartitionSpec("core"),) * n_outs,
                  check_rep=False),
        keep_unused=True)
    concat_in = [np.concatenate([np.asarray(m[n]) for m in in_maps], axis=0)
                 for n in in_names]
    concat_zero = [np.zeros((N_CORES * z.shape[0], *z.shape[1:]), z.dtype)
                   for z in zero_outs]
    outs = fn(*concat_in, *concat_zero)
    return [
        {n: np.asarray(outs[i]).reshape(N_CORES, *out_avals[i].shape)[c]
         for i, n in enumerate(out_names)}
        for c in range(N_CORES)
    ], fn, concat_in, concat_zero, out_names, out_avals


def _prepare(x_src, x_dst, e, W, b):
    """Host-side sharding prep. Returns per-core in_maps + assembly info."""
    src = e[0].astype(np.int64)
    dst = e[1].astype(np.int64)
    order = np.lexsort((src, dst))       # by dst, then src (chunks contiguous)
    src_s = src[order]
    dst_s = dst[order]
    ck = src_s // CH
    inch = (src_s % CH).astype(np.int16)
    cnt4 = np.bincount(dst_s * NCH + ck, minlength=N_DST * NCH) \
        .reshape(N_DST, NCH)
    start4 = np.concatenate([[0], np.cumsum(cnt4.ravel())])[:-1] \
        .reshape(N_DST, NCH)
    deg_all = cnt4.sum(axis=1)

    # padded striped src table (shared by all cores)
    xtab = np.empty((NCH * CHP, D), dtype=np.float32)
    for k in range(NCH):
        xtab[k * CHP:k * CHP + CH] = x_src[k * CH:(k + 1) * CH]
        xtab[k * CHP + CH] = BIG

    # cross-core dst assignment: global lex sort of per-chunk count tuples,
    # deal every-8th to a core (cores get near-identical count profiles, so
    # the shared max-over-cores schedule costs almost nothing), then order
    # slots within a core by max per-chunk count (minimizes sum_k max_p cnt
    # within each 128-slot word; ~16% fewer gather columns than per-range
    # assignment)
    alllex = np.lexsort((-cnt4[:, 3], -cnt4[:, 2], -cnt4[:, 1], -cnt4[:, 0]))
    cores = []
    Rwk = np.zeros((WORDS, NCH), dtype=np.int64)
    for c in range(N_CORES):
        ids = alllex[c::N_CORES]
        pi = np.argsort(-cnt4[ids].max(axis=1), kind="stable")
        gdst = ids[pi]                     # absolute dst id per slot
        cnt_s = np.zeros((SLOTS, NCH), dtype=np.int64)
        cnt_s[:DST_PER_CORE] = cnt4[gdst]
        st_s = np.zeros((SLOTS, NCH), dtype=np.int64)
        st_s[:DST_PER_CORE] = start4[gdst]
        Rwk = np.maximum(Rwk, cnt_s.reshape(WORDS, 128, NCH).max(axis=1))
        cores.append(dict(gdst=gdst, cnt_s=cnt_s, st_s=st_s))

    cols, calls, runs = _schedule(Rwk)
    ncol = len(cols)
    p128 = np.arange(128)

    in_maps = []
    for c in range(N_CORES):
        cc = cores[c]
        cnt_s, st_s = cc["cnt_s"], cc["st_s"]
        idx_lin = np.empty(ncol * 128, dtype=np.int16)
        j = 0
        for k in range(NCH):
            Rk = Rwk[:, k]
            rmax = int(Rk.max()) if len(Rk) else 0
            for r in range(rmax):
                ws = np.where(Rk > r)[0]
                if ws.size == 0:
                    continue
                slots = (ws[:, None] * 128 + p128[None, :]).ravel()
                cnt = cnt_s[slots, k]
                st = st_s[slots, k]
                rr = np.minimum(r, np.maximum(cnt - 1, 0))
                pos = np.minimum(st + rr, len(inch) - 1)
                val = np.where(cnt > 0, inch[pos], np.int16(CH))
                idx_lin[j * 128:(j + ws.size) * 128] = val.astype(np.int16)
                j += ws.size
        assert j == ncol
        wrapped = np.tile(idx_lin.reshape(-1, 16).T, (8, 1))
        xdT = np.zeros((D, SLOTS), dtype=np.float32)
        xdT[:, :DST_PER_CORE] = x_dst[cc["gdst"]].T
        in_maps.append({
            "xtab": xtab,
            "xdT": xdT,
            "idx": np.ascontiguousarray(wrapped),
            "w_in": np.ascontiguousarray(W),
            "b_in": np.ascontiguousarray(b.reshape(D, 1)),
        })
    return in_maps, cores, Rwk, deg_all


_CACHE = {}
_LAST = None  # (fn, concat_in, concat_zero) from the most recent call
_LAST_PREP = None  # (in_maps, Rwk) from the most recent call


def kernel(x_src, x_dst, e, W, b):
    x_src = np.asarray(x_src, dtype=np.float32)
    x_dst = np.asarray(x_dst, dtype=np.float32)
    e = np.asarray(e)
    W = np.asarray(W, dtype=np.float32)
    b = np.asarray(b, dtype=np.float32)

    in_maps, cores, Rwk, deg_all = _prepare(x_src, x_dst, e, W, b)

    key = (Rwk.tobytes(), 1)
    if key not in _CACHE:
        _CACHE[key] = _build_program(Rwk)
    nc = _CACHE[key]

    results, fn, ci, cz, on, oa = _run_spmd(nc, in_maps)
    global _LAST, _LAST_PREP
    _LAST = (fn, ci, cz)
    _LAST_PREP = (in_maps, Rwk)

    out = np.empty((N_DST, D), dtype=np.float32)
    for c in range(N_CORES):
        outT = results[c]["outT"]                     # [D, SLOTS]
        out[cores[c]["gdst"]] = outT[:, :DST_PER_CORE].T

    # exact host patch for degree-0 dsts (empty segments -> maxes = 0)
    z = np.where(deg_all == 0)[0]
    if z.size:
        h = x_dst[z] @ W[:D] + b
        h = np.where(h > 0, h, LEAKY * h)
        out[z] = x_dst[z] + h
    return out



# revision 13
# speedup vs baseline: 605.3126x; 2.2870x over previous
"""BipartiteResMRConv on 8 Trainium2 NeuronCores (Bass/Tile).

Math: out = x_dst + LeakyReLU(concat([x_dst, maxes]) @ W + b), where
maxes[d] = max over edges (s,d) of (x_dst[d] - x_src[s]) = x_dst[d] - segmin[d],
segmin[d] = min over edges of x_src[s]  (empty d -> maxes = 0).

Sharding: dsts are partitioned across 8 cores (12500 each). Per core, dsts are
sorted by degree (descending) into 12544 slots; slot j lives at SBUF partition
j%128, word j//128 of a [128, 98*128] f32 accumulator (initialized to +BIG).

The gather uses the custom SWDGE dma_gather instruction (int16 indices, fast
CounterMachine descriptor generation) instead of per-round indirect DMAs.
Because idx is int16 (<=32767) and rows are 512B, x_src is split into 4 chunks
of 25000 rows; each chunk's stripe in a host-built table gets one extra +BIG
padding row. Columns of 128 gathered rows are chunk-pure: for chunk k, round r,
word w (with R[w,k] > r), partition p receives the r-th edge of slot (w,p)
whose src lies in chunk k (repeating an edge or pointing at the +BIG row as
padding). Gathered column tiles are min-folded into the accumulator with wide
DVE ops covering runs of consecutive words.

The accumulator is PE-transposed to feature-major, combined with the
host-pre-transposed x_dst, pushed through the 2-tile fp32 matmul (W resident),
LeakyReLU+bias on ACT, residual add on DVE, and written out feature-major.
The host inverse-permutes the output and patches degree-0 dsts exactly.
"""
import numpy as np
from contextlib import ExitStack

import jax
from jax.sharding import Mesh, PartitionSpec
from jax.experimental.shard_map import shard_map

from concourse import bass, bacc, tile, mybir
from concourse.bass2jax import install_neuronx_cc_hook, _bass_exec_p, partition_id_tensor
from concourse.masks import make_identity

N_SRC = 100000
N_DST = 100000
N_EDGES = 800000
D = 128
N_CORES = 8
DST_PER_CORE = N_DST // N_CORES          # 12500
SLOTS = 12544                            # ceil(12500/128)*128
WORDS = SLOTS // 128                     # 98
LEAKY = 0.01
CHUNK_W = 4                              # words per MLP chunk (512 dsts)
NCH = 4                                  # src chunks (int16 idx limit)
CH = N_SRC // NCH                        # 25000 rows per chunk
CHP = CH + 1                             # stripe rows incl. +BIG pad row
TCOLS = 8                                # gathered columns per dma_gather call
# (8*128 = 1024 idxs -> 65 descriptors per SDMA engine, under the ~128-desc
#  per-engine ring capacity; larger calls serialize descriptor generation
#  against draining at HBM round-trip latency and run ~10x slower)
BIG = np.float32(1e30)
NQUEUES = 4                              # SWDGE queues (1-4); gathers round-robin


def _schedule(Rwk):
    """Column schedule from per-(word, chunk) round counts [WORDS, NCH].

    Returns cols [(k, w, r)], calls [(k, col0, ncols)], runs_per_call
    [[(j_in_call, w0, nwords)]].
    """
    cols = []
    calls = []
    for k in range(NCH):
        c0k = len(cols)
        Rk = Rwk[:, k]
        rmax = int(Rk.max()) if len(Rk) else 0
        for r in range(rmax):
            for w in range(WORDS):
                if Rk[w] > r:
                    cols.append((k, w, r))
        nk = len(cols) - c0k
        for t in range(0, nk, TCOLS):
            calls.append((k, c0k + t, min(TCOLS, nk - t)))
    runs_per_call = []
    for (k, c0, n) in calls:
        runs = []
        j = 0
        while j < n:
            w0 = cols[c0 + j][1]
            m = 1
            while j + m < n and cols[c0 + j + m][1] == w0 + m:
                m += 1
            runs.append((j, w0, m))
            j += m
        runs_per_call.append(runs)
    return cols, calls, runs_per_call


def _build_program(Rwk, loops=1):
    """Build the bass program. loops>1 wraps the whole per-iteration body in
    a hardware For_i loop (each iteration is a complete kernel execution,
    separated by the loop's all-engine barrier) — used by test.py to amortize
    the dispatch round-trip when timing."""
    cols, calls, runs_per_call = _schedule(Rwk)
    ncol = len(cols)
    nc = bacc.Bacc("TRN2", target_bir_lowering=False, debug=False,
                   num_devices=N_CORES, dynamic_dma_scratch_size=32768,
                   num_swdge_queues=NQUEUES)
    f32 = mybir.dt.float32
    xtab = nc.dram_tensor("xtab", [NCH * CHP, D], f32, kind="ExternalInput").ap()
    xdT = nc.dram_tensor("xdT", [D, SLOTS], f32, kind="ExternalInput").ap()
    idx = nc.dram_tensor("idx", [128, max(ncol * 8, 8)], mybir.dt.int16,
                         kind="ExternalInput").ap()
    w_in = nc.dram_tensor("w_in", [2 * D, D], f32, kind="ExternalInput").ap()
    b_in = nc.dram_tensor("b_in", [D, 1], f32, kind="ExternalInput").ap()
    outT = nc.dram_tensor("outT", [D, SLOTS], f32, kind="ExternalOutput").ap()

    with tile.TileContext(nc) as tc, ExitStack() as ctx:
        pool = ctx.enter_context(tc.tile_pool(name="pool", bufs=1))
        ipool = ctx.enter_context(tc.tile_pool(name="ipool", bufs=8))
        gpool = ctx.enter_context(tc.tile_pool(name="gpool", bufs=8))
        cpool = ctx.enter_context(tc.tile_pool(name="cpool", bufs=3))
        tpsum = ctx.enter_context(tc.tile_pool(name="tpsum", bufs=3, space="PSUM"))
        mpsum = ctx.enter_context(tc.tile_pool(name="mpsum", bufs=3, space="PSUM"))

        xdT_t = pool.tile([D, SLOTS], f32)
        nc.sync.dma_start(out=xdT_t[:], in_=xdT[:])
        wa = pool.tile([D, D], f32)
        nc.sync.dma_start(out=wa[:], in_=w_in[0:D, :])
        wb = pool.tile([D, D], f32)
        nc.sync.dma_start(out=wb[:], in_=w_in[D:2 * D, :])
        b_t = pool.tile([D, 1], f32)
        nc.sync.dma_start(out=b_t[:], in_=b_in[:])
        ident = pool.tile([128, 128], f32)
        make_identity(nc, ident[:])
        acc = pool.tile([128, SLOTS], f32)

        def body():
            nc.vector.memset(acc[:], float(BIG))

            for ci, ((kk, c0, n), runs) in enumerate(zip(calls, runs_per_call)):
                idx_t = ipool.tile([128, TCOLS * 8], mybir.dt.int16, tag="idx")
                nc.sync.dma_start(out=idx_t[:, :n * 8],
                                  in_=idx[:, c0 * 8:(c0 + n) * 8])
                g = gpool.tile([128, TCOLS, D], f32, tag="g")
                nc.gpsimd.dma_gather(
                    g[:, :n, :], xtab[kk * CHP:(kk + 1) * CHP, :],
                    idx_t[:, :n * 8], n * 128, n * 128, D,
                    single_packet=False,  # HW packets cap at 64 descs/engine
                    queue_num=ci % NQUEUES)
                for (j, w0, m) in runs:
                    sl = slice(w0 * 128, (w0 + m) * 128)
                    nc.vector.tensor_tensor(
                        out=acc[:, sl], in0=acc[:, sl],
                        in1=g[:, j:j + m, :].rearrange("p a b -> p (a b)"),
                        op=mybir.AluOpType.min)

            # MLP in chunks of CHUNK_W words (512 dst columns)
            for c in range(WORDS // CHUNK_W + (1 if WORDS % CHUNK_W else 0)):
                w0 = c * CHUNK_W
                nwc = min(CHUNK_W, WORDS - w0)
                ncolw = nwc * 128
                csl = slice(w0 * 128, w0 * 128 + ncolw)
                accT = tpsum.tile([128, CHUNK_W * 128], f32, space="PSUM",
                                  tag="accT")
                for i in range(nwc):
                    nc.tensor.transpose(
                        out=accT[:, i * 128:(i + 1) * 128],
                        in_=acc[:, (w0 + i) * 128:(w0 + i + 1) * 128],
                        identity=ident[:])
                maxT = cpool.tile([128, CHUNK_W * 128], f32, tag="maxT")
                nc.vector.tensor_tensor(out=maxT[:, :ncolw], in0=xdT_t[:, csl],
                                        in1=accT[:, :ncolw],
                                        op=mybir.AluOpType.subtract)
                hp = mpsum.tile([128, CHUNK_W * 128], f32, space="PSUM", tag="hp")
                nc.tensor.matmul(out=hp[:, :ncolw], lhsT=wa[:], rhs=xdT_t[:, csl],
                                 start=True, stop=False)
                nc.tensor.matmul(out=hp[:, :ncolw], lhsT=wb[:],
                                 rhs=maxT[:, :ncolw], start=False, stop=True)
                h = cpool.tile([128, CHUNK_W * 128], f32, tag="h")
                nc.scalar.activation(out=h[:, :ncolw], in_=hp[:, :ncolw],
                                     func=mybir.ActivationFunctionType.Lrelu,
                                     bias=b_t[:], scale=1.0, alpha=LEAKY)
                res = cpool.tile([128, CHUNK_W * 128], f32, tag="res")
                nc.vector.tensor_tensor(out=res[:, :ncolw], in0=xdT_t[:, csl],
                                        in1=h[:, :ncolw], op=mybir.AluOpType.add)
                nc.sync.dma_start(out=outT[:, csl], in_=res[:, :ncolw])

        if loops > 1:
            with tc.For_i(0, loops, 1):
                body()
        else:
            body()
    nc.compile()
    return nc


def _run_spmd(nc, in_maps):
    install_neuronx_cc_hook()
    partition_name = nc.partition_id_tensor.name if nc.partition_id_tensor else None
    in_names, out_names, out_avals, zero_outs = [], [], [], []
    for alloc in nc.m.functions[0].allocations:
        if not isinstance(alloc, mybir.MemoryLocationSet):
            continue
        name = alloc.memorylocations[0].name
        if alloc.kind == "ExternalInput":
            if name != partition_name:
                in_names.append(name)
        elif alloc.kind == "ExternalOutput":
            shape = tuple(alloc.tensor_shape)
            dtype = mybir.dt.np(alloc.dtype)
            out_names.append(name)
            out_avals.append(jax.core.ShapedArray(shape, dtype))
            zero_outs.append(np.zeros(shape, dtype))
    n_params = len(in_names)
    n_outs = len(out_avals)
    all_in = list(in_names) + list(out_names)
    if partition_name is not None:
        all_in.append(partition_name)

    def _body(*args):
        operands = list(args)
        if partition_name is not None:
            operands.append(partition_id_tensor())
        return tuple(_bass_exec_p.bind(
            *operands, out_avals=tuple(out_avals), in_names=tuple(all_in),
            out_names=tuple(out_names), lowering_input_output_aliases=(),
            sim_require_finite=True, sim_require_nnan=True, nc=nc))

    devices = jax.devices()[:N_CORES]
    mesh = Mesh(np.asarray(devices), ("core",))
    fn = jax.jit(
        shard_map(_body, mesh=mesh,
                  in_specs=(PartitionSpec("core"),) * (n_params + n_outs),
                  out_specs=(PartitionSpec("core"),) * n_outs,
                  check_rep=False),
        keep_unused=True)
    concat_in = [np.concatenate([np.asarray(m[n]) for m in in_maps], axis=0)
                 for n in in_names]
    concat_zero = [np.zeros((N_CORES * z.shape[0], *z.shape[1:]), z.dtype)
                   for z in zero_outs]
    outs = fn(*concat_in, *concat_zero)
    return [
        {n: np.asarray(outs[i]).reshape(N_CORES, *out_avals[i].shape)[c]
         for i, n in enumerate(out_names)}
        for c in range(N_CORES)
    ], fn, concat_in, concat_zero, out_names, out_avals


def _prepare(x_src, x_dst, e, W, b):
    """Host-side sharding prep. Returns per-core in_maps + assembly info."""
    src = e[0].astype(np.int64)
    dst = e[1].astype(np.int64)
    order = np.lexsort((src, dst))       # by dst, then src (chunks contiguous)
    src_s = src[order]
    dst_s = dst[order]
    ck = src_s // CH
    inch = (src_s % CH).astype(np.int16)
    cnt4 = np.bincount(dst_s * NCH + ck, minlength=N_DST * NCH) \
        .reshape(N_DST, NCH)
    start4 = np.concatenate([[0], np.cumsum(cnt4.ravel())])[:-1] \
        .reshape(N_DST, NCH)
    deg_all = cnt4.sum(axis=1)

    # padded striped src table (shared by all cores)
    xtab = np.empty((NCH * CHP, D), dtype=np.float32)
    for k in range(NCH):
        xtab[k * CHP:k * CHP + CH] = x_src[k * CH:(k + 1) * CH]
        xtab[k * CHP + CH] = BIG

    # cross-core dst assignment: global lex sort of per-chunk count tuples,
    # deal every-8th to a core (cores get near-identical count profiles, so
    # the shared max-over-cores schedule costs almost nothing), then order
    # slots within a core by max per-chunk count (minimizes sum_k max_p cnt
    # within each 128-slot word; ~16% fewer gather columns than per-range
    # assignment)
    alllex = np.lexsort((-cnt4[:, 3], -cnt4[:, 2], -cnt4[:, 1], -cnt4[:, 0]))
    cores = []
    Rwk = np.zeros((WORDS, NCH), dtype=np.int64)
    for c in range(N_CORES):
        ids = alllex[c::N_CORES]
        pi = np.argsort(-cnt4[ids].max(axis=1), kind="stable")
        gdst = ids[pi]                     # absolute dst id per slot
        cnt_s = np.zeros((SLOTS, NCH), dtype=np.int64)
        cnt_s[:DST_PER_CORE] = cnt4[gdst]
        st_s = np.zeros((SLOTS, NCH), dtype=np.int64)
        st_s[:DST_PER_CORE] = start4[gdst]
        Rwk = np.maximum(Rwk, cnt_s.reshape(WORDS, 128, NCH).max(axis=1))
        cores.append(dict(gdst=gdst, cnt_s=cnt_s, st_s=st_s))

    cols, calls, runs = _schedule(Rwk)
    ncol = len(cols)
    p128 = np.arange(128)

    in_maps = []
    for c in range(N_CORES):
        cc = cores[c]
        cnt_s, st_s = cc["cnt_s"], cc["st_s"]
        idx_lin = np.empty(ncol * 128, dtype=np.int16)
        j = 0
        for k in range(NCH):
            Rk = Rwk[:, k]
            rmax = int(Rk.max()) if len(Rk) else 0
            for r in range(rmax):
                ws = np.where(Rk > r)[0]
                if ws.size == 0:
                    continue
                slots = (ws[:, None] * 128 + p128[None, :]).ravel()
                cnt = cnt_s[slots, k]
                st = st_s[slots, k]
                rr = np.minimum(r, np.maximum(cnt - 1, 0))
                pos = np.minimum(st + rr, len(inch) - 1)
                val = np.where(cnt > 0, inch[pos], np.int16(CH))
                idx_lin[j * 128:(j + ws.size) * 128] = val.astype(np.int16)
                j += ws.size
        assert j == ncol
        wrapped = np.tile(idx_lin.reshape(-1, 16).T, (8, 1))
        xdT = np.zeros((D, SLOTS), dtype=np.float32)
        xdT[:, :DST_PER_CORE] = x_dst[cc["gdst"]].T
        in_maps.append({
            "xtab": xtab,
            "xdT": xdT,
            "idx": np.ascontiguousarray(wrapped),
            "w_in": np.ascontiguousarray(W),
            "b_in": np.ascontiguousarray(b.reshape(D, 1)),
        })
    return in_maps, cores, Rwk, deg_all


_CACHE = {}
_LAST = None  # (fn, concat_in, concat_zero) from the most recent call
_LAST_PREP = None  # (in_maps, Rwk) from the most recent call


def kernel(x_src, x_dst, e, W, b):
    x_src = np.asarray(x_src, dtype=np.float32)
    x_dst = np.asarray(x_dst, dtype=np.float32)
    e = np.asarray(e)
    W = np.asarray(W, dtype=np.float32)
    b = np.asarray(b, dtype=np.float32)

    in_maps, cores, Rwk, deg_all = _prepare(x_src, x_dst, e, W, b)

    key = (Rwk.tobytes(), 1)
    if key not in _CACHE:
        _CACHE[key] = _build_program(Rwk)
    nc = _CACHE[key]

    results, fn, ci, cz, on, oa = _run_spmd(nc, in_maps)
    global _LAST, _LAST_PREP
    _LAST = (fn, ci, cz)
    _LAST_PREP = (in_maps, Rwk)

    out = np.empty((N_DST, D), dtype=np.float32)
    for c in range(N_CORES):
        outT = results[c]["outT"]                     # [D, SLOTS]
        out[cores[c]["gdst"]] = outT[:, :DST_PER_CORE].T

    # exact host patch for degree-0 dsts (empty segments -> maxes = 0)
    z = np.where(deg_all == 0)[0]
    if z.size:
        h = x_dst[z] @ W[:D] + b
        h = np.where(h > 0, h, LEAKY * h)
        out[z] = x_dst[z] + h
    return out

